# revision 8
# baseline (speedup 1.0000x reference)
"""ChannelDeconv (training-mode forward, C == block == 64) on 8 TRN2 NeuronCores.

Reference math (see problem):
    x: (32, 64, 128, 128) f32, NCHW
    x1    = x.transpose(1,0,2,3).reshape(64, N*H*W)        # [B, L], L = 524288
    x1_s  = x1[:, ::9]                                     # 58255 sampled cols
    mean  = x1_s.mean(-1)                                  # [B]
    cov   = x1_s @ x1_s.T / n_s + 0.01*I                   # [B, B]
    D     = newton_schulz_isqrt(cov, 5)
    y     = D @ (x1 - mean)  -> reshape back to NCHW

Sharding: data-parallel over N. Core k owns images [4k, 4k+4). Each core:
  - receives its 4 images pre-stacked as [128, 32768] (partitions 0:64 =
    channels of images 0,1; partitions 64:128 = channels of images 2,3)
  - receives its share of the *global* stride-9 sample set, host-gathered
    and pre-transposed into [128, 57*65] chunk-major layout (col 64 of each
    chunk row is a 0/1 validity flag; it yields the column sums + count in
    the same Gram matmul)
  - computes the partial Gram [65,65] on the PE, AllReduces it (16.9KB),
    runs Newton-Schulz redundantly, applies the block-diag 128x128
    whitening matmul locally and streams the output back.
"""

import importlib.util
import os
import sys

if importlib.util.find_spec("concourse") is None:
    for _p in ("/opt/trn_rl_repo", os.path.expanduser("~/.axon_site/_ro/trn_rl_repo")):
        if os.path.isdir(_p) and _p not in sys.path:
            sys.path.insert(0, _p)

import numpy as np

N, C, H, W = 32, 64, 128, 128
HW = H * W               # 16384
B = 64                   # whitening block / channel count
STRIDE2 = 9              # sampling stride**2
EPS = 0.01
N_ITER = 5
CORES = 8
NL = N // CORES          # images per core = 4
LPC = NL * HW            # columns per core = 65536
WIDE = 2 * HW            # stacked free dim = 32768
NS_TOT = (N * HW + STRIDE2 - 1) // STRIDE2   # 58255 global samples
NCHUNK = 57              # sample chunks of 128 rows per core (57*128=7296)
MCOL = B + 1             # 64 channels + validity column

_cached = {}


def _build_nc():
    import concourse.mybir as mybir
    import concourse.tile as tile
    from concourse import bacc

    f32 = mybir.dt.float32
    # Bacc (not raw Bass): its compile() pipeline legalizes sync waits
    # (move_matmul_waits_to_ldweights + generate_event_semaphores) to the
    # 1-wait-per-instruction hardware budget.
    nc = bacc.Bacc(None, num_devices=CORES)

    xp = nc.declare_dram_parameter("xp", [128, WIDE], f32, isOutput=False)
    xst = nc.declare_dram_parameter("xst", [128, NCHUNK * MCOL], f32, isOutput=False)
    eye_in = nc.declare_dram_parameter("eye", [B, B], f32, isOutput=False)
    out_ext = nc.declare_dram_parameter("out", [128, WIDE], f32, isOutput=True)

    cc_in = nc.dram_tensor("cc_in", [MCOL, MCOL], f32)
    cc_out = nc.dram_tensor("cc_out", [MCOL, MCOL], f32)

    inv_cnt = 1.0 / float(NS_TOT)

    with tile.TileContext(nc) as tc:
        with (
            tc.tile_pool(name="big", bufs=1) as big,
            tc.tile_pool(name="stage", bufs=1) as stage,
            tc.tile_pool(name="smalls", bufs=1) as smalls,
            tc.tile_pool(name="nsp", bufs=2) as nsp,
            tc.tile_pool(name="psg", bufs=1, space="PSUM") as psg,
            tc.tile_pool(name="pss", bufs=3, space="PSUM") as pss,
            tc.tile_pool(name="psw", bufs=4, space="PSUM") as psw,
            tc.tile_pool(name="outs", bufs=3) as outs,
        ):
            # ---- input DMAs -------------------------------------------------
            xst_sb = stage.tile([128, NCHUNK * MCOL], f32)
            nc.sync.dma_start(out=xst_sb[:, :], in_=xst[:, :])
            eye_sb = smalls.tile([B, B], f32)
            nc.sync.dma_start(out=eye_sb[:, :], in_=eye_in[:, :])
            S = big.tile([128, WIDE], f32)
            NSPLIT = 8
            csz = WIDE // NSPLIT
            for i in range(NSPLIT):
                nc.sync.dma_start(
                    out=S[:, i * csz:(i + 1) * csz],
                    in_=xp[:, i * csz:(i + 1) * csz],
                )

            # ---- partial Gram (and column sums via the validity column) -----
            g_ps = psg.tile([MCOL, MCOL], f32)
            for i in range(NCHUNK):
                chunk = xst_sb[:, i * MCOL:(i + 1) * MCOL]
                nc.tensor.matmul(
                    g_ps[:, :], lhsT=chunk, rhs=chunk,
                    start=(i == 0), stop=(i == NCHUNK - 1),
                )
            stats_sb = smalls.tile([MCOL, MCOL], f32)
            nc.vector.tensor_copy(out=stats_sb[:, :], in_=g_ps[:, :])

            # ---- AllReduce the [65,65] stats --------------------------------
            # (gpsimd/SWDGE: these tiny bounce DMAs can legitimately need two
            # sync waits; the HWDGE DIRECT2D encoding only fits one)
            nc.gpsimd.dma_start(out=cc_in[:, :], in_=stats_sb[:, :])
            nc.gpsimd.collective_compute(
                "AllReduce",
                mybir.AluOpType.add,
                replica_groups=[list(range(CORES))],
                ins=[cc_in[:, :].opt()],
                outs=[cc_out[:, :].opt()],
            )
            ar_sb = smalls.tile([MCOL, MCOL], f32)
            nc.gpsimd.dma_start(out=ar_sb[:, :], in_=cc_out[:, :])

            # ---- cov = G/n + eps*I ------------------------------------------
            # engine discipline: every small elementwise/copy op goes to DVE
            # (vector) and only the two Sqrts to ACT -- each consumer then
            # depends on at most 2 distinct semaphores (the per-opcode
            # sync-wait budget is tiny: HWDGE DMA fits 1 wait, DVE ops 2).
            covt = smalls.tile([B, B], f32)
            nc.vector.tensor_scalar_mul(out=covt[:, :], in0=ar_sb[0:B, 0:B],
                                        scalar1=inv_cnt)
            eps_eye = smalls.tile([B, B], f32)
            nc.vector.tensor_scalar_mul(out=eps_eye[:, :], in0=eye_sb[:, :],
                                        scalar1=EPS)
            cov = smalls.tile([B, B], f32)
            nc.vector.tensor_add(out=cov[:, :], in0=covt[:, :], in1=eps_eye[:, :])
            eye3 = smalls.tile([B, B], f32)
            nc.vector.tensor_scalar_mul(out=eye3[:, :], in0=eye_sb[:, :],
                                        scalar1=3.0)
            ones64 = smalls.tile([B, B], f32)
            nc.vector.memset(ones64[:, :], 1.0)

            # ---- Frobenius norm: normA^2 broadcast to all partitions --------
            sq = smalls.tile([B, B], f32)
            nc.vector.tensor_mul(out=sq[:, :], in0=cov[:, :], in1=cov[:, :])
            rsum = smalls.tile([B, 1], f32)
            nc.vector.reduce_sum(out=rsum[:, :], in_=sq[:, :], axis=mybir.AxisListType.X)
            nsq_ps = pss.tile([B, 1], f32, tag="pss")
            nc.tensor.matmul(nsq_ps[:, :], lhsT=ones64[:, :], rhs=rsum[:, :],
                             start=True, stop=True)
            normA = smalls.tile([B, 1], f32)
            nc.scalar.activation(out=normA[:, :], in_=nsq_ps[:, :],
                                 func=mybir.ActivationFunctionType.Sqrt)
            rnorm = smalls.tile([B, 1], f32)
            nc.vector.reciprocal(out=rnorm[:, :], in_=normA[:, :])
            rqnorm = smalls.tile([B, 1], f32)
            nc.scalar.activation(out=rqnorm[:, :], in_=rnorm[:, :],
                                 func=mybir.ActivationFunctionType.Sqrt)

            # ---- Newton-Schulz (everything symmetric, so lhsT == operand) ---
            Y = smalls.tile([B, B], f32)
            nc.vector.tensor_scalar_mul(out=Y[:, :], in0=cov[:, :], scalar1=rnorm[:, :])
            Z = None
            for it in range(N_ITER):
                T = nsp.tile([B, B], f32, tag="T", name=f"T{it}")
                if it == 0:
                    # Z_0 = I  ->  T = 3I - Y
                    nc.vector.tensor_sub(out=T[:, :], in0=eye3[:, :], in1=Y[:, :])
                else:
                    zy_ps = pss.tile([B, B], f32, tag="pss", name=f"zy{it}")
                    nc.tensor.matmul(zy_ps[:, :], lhsT=Z[:, :], rhs=Y[:, :],
                                     start=True, stop=True)
                    nc.vector.tensor_sub(out=T[:, :], in0=eye3[:, :], in1=zy_ps[:, :])
                yn_ps = pss.tile([B, B], f32, tag="pss", name=f"yn{it}")
                nc.tensor.matmul(yn_ps[:, :], lhsT=Y[:, :], rhs=T[:, :],
                                 start=True, stop=True)
                Yn = nsp.tile([B, B], f32, tag="Y", name=f"Y{it}")
                nc.vector.tensor_scalar_mul(out=Yn[:, :], in0=yn_ps[:, :], scalar1=0.5)
                Zn = nsp.tile([B, B], f32, tag="Z", name=f"Z{it}")
                if it == 0:
                    nc.vector.tensor_scalar_mul(out=Zn[:, :], in0=T[:, :], scalar1=0.5)
                else:
                    zn_ps = pss.tile([B, B], f32, tag="pss", name=f"zn{it}")
                    nc.tensor.matmul(zn_ps[:, :], lhsT=T[:, :], rhs=Z[:, :],
                                     start=True, stop=True)
                    nc.vector.tensor_scalar_mul(out=Zn[:, :], in0=zn_ps[:, :],
                                                scalar1=0.5)
                Y, Z = Yn, Zn

            deconv = smalls.tile([B, B], f32)
            nc.vector.tensor_scalar_mul(out=deconv[:, :], in0=Z[:, :],
                                        scalar1=rqnorm[:, :])

            # ---- stacked bias dm2 = [D@mean; D@mean] via partition-offset
            # matmuls (no cross-partition SBUF DMAs needed)
            mean_sb = smalls.tile([B, 1], f32)
            nc.vector.tensor_scalar_mul(out=mean_sb[:, :], in0=ar_sb[0:B, B:B + 1],
                                        scalar1=inv_cnt)
            dm2_ps = pss.tile([128, 1], f32, tag="pss")
            nc.tensor.matmul(dm2_ps[0:B, :], lhsT=deconv[:, :], rhs=mean_sb[:, :],
                             start=True, stop=True)
            nc.tensor.matmul(dm2_ps[B:128, :], lhsT=deconv[:, :], rhs=mean_sb[:, :],
                             start=True, stop=True)
            dm2 = smalls.tile([128, 1], f32)
            nc.vector.tensor_copy(out=dm2[:, :], in_=dm2_ps[:, :])

            # ---- block-diagonal [[D,0],[0,D]]: D @ I into both diagonal
            # PSUM quadrants (out base-partition 0 and 64), off-diagonal
            # quadrants stay zero in a pre-zeroed SBUF tile
            dblk_ps = pss.tile([128, 128], f32, tag="pss")
            nc.tensor.matmul(dblk_ps[0:B, 0:B], lhsT=deconv[:, :], rhs=eye_sb[:, :],
                             start=True, stop=True)
            nc.tensor.matmul(dblk_ps[B:128, B:128], lhsT=deconv[:, :],
                             rhs=eye_sb[:, :], start=True, stop=True)
            dblk = smalls.tile([128, 128], f32)
            nc.vector.memset(dblk[:, :], 0.0)
            nc.vector.tensor_copy(out=dblk[0:B, 0:B], in_=dblk_ps[0:B, 0:B])
            nc.vector.tensor_copy(out=dblk[B:128, B:128], in_=dblk_ps[B:128, B:128])

            # ---- whitening: y = Dblk @ S - dm2 ------------------------------
            NBLK = 512                      # one PSUM bank
            GRP = 4                         # blocks per output DMA (1 MiB)
            for jo in range(WIDE // (NBLK * GRP)):
                y_sb = outs.tile([128, NBLK * GRP], f32, tag="y", name=f"y{jo}")
                for ji in range(GRP):
                    j = jo * GRP + ji
                    w_ps = psw.tile([128, NBLK], f32, tag="w", name=f"w{j}")
                    nc.tensor.matmul(
                        w_ps[:, :], lhsT=dblk[:, :],
                        rhs=S[:, j * NBLK:(j + 1) * NBLK],
                        start=True, stop=True,
                    )
                    nc.vector.tensor_scalar(
                        out=y_sb[:, ji * NBLK:(ji + 1) * NBLK],
                        in0=w_ps[:, :], scalar1=dm2[:, :], scalar2=None,
                        op0=mybir.AluOpType.subtract,
                    )
                nc.sync.dma_start(
                    out=out_ext[:, jo * NBLK * GRP:(jo + 1) * NBLK * GRP],
                    in_=y_sb[:, :],
                )

    nc.finalize()
    return nc


def _shard_inputs(x):
    """Build per-core input maps from the full (32,64,128,128) f32 tensor."""
    x = np.ascontiguousarray(x, dtype=np.float32)
    xr = x.reshape(N, C, HW)

    # global stride-9 sample gather, already transposed to [n_samples, 64]
    ls = np.arange(0, N * HW, STRIDE2, dtype=np.int64)
    ns_idx = ls // HW
    hw_idx = ls % HW
    xs_all = xr[ns_idx, :, hw_idx]          # [58255, 64]

    eye = np.eye(B, dtype=np.float32)
    in_maps = []
    for k in range(CORES):
        # stacked main input [128, 32768]
        x4 = x[NL * k:NL * (k + 1)].reshape(2, 2, C, HW)
        xp = np.ascontiguousarray(
            x4.transpose(0, 2, 1, 3).reshape(128, WIDE))

        # this core's slice of the global sample set
        s0 = -(-(LPC * k) // STRIDE2)        # ceil
        s1 = -(-(LPC * (k + 1)) // STRIDE2)
        m = s1 - s0
        xst = np.zeros((NCHUNK * 128, MCOL), dtype=np.float32)
        xst[:m, :B] = xs_all[s0:s1]
        xst[:m, B] = 1.0
        xst = np.ascontiguousarray(
            xst.reshape(NCHUNK, 128, MCOL).transpose(1, 0, 2)
            .reshape(128, NCHUNK * MCOL))

        in_maps.append({"xp": xp, "xst": xst, "eye": eye})
    return in_maps


def _unshard_output(results):
    y = np.empty((N, C, H, W), dtype=np.float32)
    for k in range(CORES):
        o = results[k]["out"].reshape(2, C, 2, HW)
        y[NL * k:NL * (k + 1)] = (
            o.transpose(0, 2, 1, 3).reshape(NL, C, H, W))
    return y


def kernel(x):
    from concourse.bass_utils import run_bass_kernel_spmd

    if "nc" not in _cached:
        _cached["nc"] = _build_nc()
    nc = _cached["nc"]

    in_maps = _shard_inputs(np.asarray(x))
    res = run_bass_kernel_spmd(nc, in_maps, core_ids=list(range(CORES)))
    _cached["last_results"] = res
    return _unshard_output(res.results)


# revision 17
# speedup vs baseline: 1.1490x; 1.1490x over previous
"""ChannelDeconv (training-mode forward, C == block == 64) on 8 TRN2 NeuronCores.

Reference math (see problem):
    x: (32, 64, 128, 128) f32, NCHW
    x1    = x.transpose(1,0,2,3).reshape(64, N*H*W)        # [B, L], L = 524288
    x1_s  = x1[:, ::9]                                     # 58255 sampled cols
    mean  = x1_s.mean(-1)                                  # [B]
    cov   = x1_s @ x1_s.T / n_s + 0.01*I                   # [B, B]
    D     = newton_schulz_isqrt(cov, 5)
    y     = D @ (x1 - mean)  -> reshape back to NCHW

Sharding: data-parallel over N. Core k owns images [4k, 4k+4). Each core:
  - receives its 4 images pre-stacked as [128, 32768] (partitions 0:64 =
    channels of images 0,1; partitions 64:128 = channels of images 2,3)
  - receives its share of the *global* stride-9 sample set, host-gathered
    and pre-transposed into [128, 57*65] chunk-major layout (col 64 of each
    chunk row is a 0/1 validity flag; it yields the column sums + count in
    the same Gram matmul)
  - computes the partial Gram [65,65] on the PE, AllReduces it (16.9KB),
    runs Newton-Schulz redundantly, applies the block-diag 128x128
    whitening matmul locally and streams the output back.
"""

import importlib.util
import os
import sys

if importlib.util.find_spec("concourse") is None:
    for _p in ("/opt/trn_rl_repo", os.path.expanduser("~/.axon_site/_ro/trn_rl_repo")):
        if os.path.isdir(_p) and _p not in sys.path:
            sys.path.insert(0, _p)

import numpy as np

N, C, H, W = 32, 64, 128, 128
HW = H * W               # 16384
B = 64                   # whitening block / channel count
STRIDE2 = 9              # sampling stride**2
EPS = 0.01
N_ITER = 5
CORES = 8
NL = N // CORES          # images per core = 4
LPC = NL * HW            # columns per core = 65536
WIDE = 2 * HW            # stacked free dim = 32768
NS_TOT = (N * HW + STRIDE2 - 1) // STRIDE2   # 58255 global samples
NCHUNK = 57              # sample chunks of 128 rows per core (57*128=7296)
MCOL = B + 1             # 64 channels + validity column

_cached = {}


def _build_nc():
    import concourse.mybir as mybir
    import concourse.tile as tile
    from concourse import bacc

    f32 = mybir.dt.float32
    f32r = mybir.dt.float32r
    # Bacc (not raw Bass): its compile() pipeline legalizes sync waits
    # (move_matmul_waits_to_ldweights + generate_event_semaphores) to the
    # 1-wait-per-instruction hardware budget.
    nc = bacc.Bacc(None, num_devices=CORES)

    # xp/S are float32r (same 4-byte layout as f32; the PE rounds on read):
    # the whitening matmul then streams at full rate instead of f32's 1/4.
    xp = nc.declare_dram_parameter("xp", [128, WIDE], f32r, isOutput=False)
    xst = nc.declare_dram_parameter("xst", [128, NCHUNK * MCOL], f32, isOutput=False)
    eye_in = nc.declare_dram_parameter("eye", [B, B], f32, isOutput=False)
    out_ext = nc.declare_dram_parameter("out", [128, WIDE], f32, isOutput=True)

    cc_in = nc.dram_tensor("cc_in", [MCOL, MCOL], f32)
    cc_out = nc.dram_tensor("cc_out", [MCOL, MCOL], f32)

    inv_cnt = 1.0 / float(NS_TOT)

    with tile.TileContext(nc) as tc:
        with (
            tc.tile_pool(name="big", bufs=1) as big,
            tc.tile_pool(name="stage", bufs=1) as stage,
            tc.tile_pool(name="smalls", bufs=1) as smalls,
            tc.tile_pool(name="nsp", bufs=2) as nsp,
            tc.tile_pool(name="psg", bufs=1, space="PSUM") as psg,
            tc.tile_pool(name="pss", bufs=3, space="PSUM") as pss,
            tc.tile_pool(name="psw", bufs=4, space="PSUM") as psw,
            tc.tile_pool(name="outs", bufs=3) as outs,
        ):
            # ---- input DMAs -------------------------------------------------
            # xst + the first two S chunks stream immediately; the remaining
            # S chunks are held back behind the AllReduce (below) so the
            # collective gets a DMA-quiet window instead of being starved
            # behind 16.8MB of queued input descriptors.
            xst_sb = stage.tile([128, NCHUNK * MCOL], f32)
            nc.sync.dma_start(out=xst_sb[:, :], in_=xst[:, :])
            eye_sb = smalls.tile([B, B], f32)
            nc.sync.dma_start(out=eye_sb[:, :], in_=eye_in[:, :])
            S = big.tile([128, WIDE], f32r)
            NSPLIT = 8
            N_EARLY = 2
            csz = WIDE // NSPLIT
            late_s_dmas = []
            for i in range(NSPLIT):
                d = nc.sync.dma_start(
                    out=S[:, i * csz:(i + 1) * csz],
                    in_=xp[:, i * csz:(i + 1) * csz],
                )
                if i >= N_EARLY:
                    late_s_dmas.append(d)

            # ---- partial Gram (and column sums via the validity column) -----
            g_ps = psg.tile([MCOL, MCOL], f32)
            for i in range(NCHUNK):
                chunk = xst_sb[:, i * MCOL:(i + 1) * MCOL]
                nc.tensor.matmul(
                    g_ps[:, :], lhsT=chunk, rhs=chunk,
                    start=(i == 0), stop=(i == NCHUNK - 1),
                )
            stats_sb = smalls.tile([MCOL, MCOL], f32)
            nc.vector.tensor_copy(out=stats_sb[:, :], in_=g_ps[:, :])

            # ---- AllReduce the [65,65] stats --------------------------------
            # (gpsimd/SWDGE: these tiny bounce DMAs can legitimately need two
            # sync waits; the HWDGE DIRECT2D encoding only fits one)
            nc.gpsimd.dma_start(out=cc_in[:, :], in_=stats_sb[:, :])
            cc = nc.gpsimd.collective_compute(
                "AllReduce",
                mybir.AluOpType.add,
                replica_groups=[list(range(CORES))],
                ins=[cc_in[:, :].opt()],
                outs=[cc_out[:, :].opt()],
            )
            ar_sb = smalls.tile([MCOL, MCOL], f32)
            nc.gpsimd.dma_start(out=ar_sb[:, :], in_=cc_out[:, :])
            # hold the bulk of the input stream until the collective is done
            if os.environ.get("KD_QUIET_WINDOW", "0") == "1":
                for d in late_s_dmas:
                    tile.add_dep_helper(d.ins, cc.ins, sync=True,
                                        reason="quiet DMA window for AllReduce")

            # ---- cov = G/n + eps*I ------------------------------------------
            # engine discipline: every small elementwise/copy op goes to DVE
            # (vector) and only the two Sqrts to ACT -- each consumer then
            # depends on at most 2 distinct semaphores (the per-opcode
            # sync-wait budget is tiny: HWDGE DMA fits 1 wait, DVE ops 2).
            covt = smalls.tile([B, B], f32)
            nc.vector.tensor_scalar_mul(out=covt[:, :], in0=ar_sb[0:B, 0:B],
                                        scalar1=inv_cnt)
            eps_eye = smalls.tile([B, B], f32)
            nc.vector.tensor_scalar_mul(out=eps_eye[:, :], in0=eye_sb[:, :],
                                        scalar1=EPS)
            cov = smalls.tile([B, B], f32)
            nc.vector.tensor_add(out=cov[:, :], in0=covt[:, :], in1=eps_eye[:, :])
            eye3 = smalls.tile([B, B], f32)
            nc.vector.tensor_scalar_mul(out=eye3[:, :], in0=eye_sb[:, :],
                                        scalar1=3.0)
            ones64 = smalls.tile([B, B], f32)
            nc.vector.memset(ones64[:, :], 1.0)

            # ---- Frobenius norm: normA^2 broadcast to all partitions --------
            sq = smalls.tile([B, B], f32)
            nc.vector.tensor_mul(out=sq[:, :], in0=cov[:, :], in1=cov[:, :])
            rsum = smalls.tile([B, 1], f32)
            nc.vector.reduce_sum(out=rsum[:, :], in_=sq[:, :], axis=mybir.AxisListType.X)
            nsq_ps = pss.tile([B, 1], f32, tag="pss")
            nc.tensor.matmul(nsq_ps[:, :], lhsT=ones64[:, :], rhs=rsum[:, :],
                             start=True, stop=True)
            normA = smalls.tile([B, 1], f32)
            nc.scalar.activation(out=normA[:, :], in_=nsq_ps[:, :],
                                 func=mybir.ActivationFunctionType.Sqrt)
            rnorm = smalls.tile([B, 1], f32)
            nc.vector.reciprocal(out=rnorm[:, :], in_=normA[:, :])
            rqnorm = smalls.tile([B, 1], f32)
            nc.scalar.activation(out=rqnorm[:, :], in_=rnorm[:, :],
                                 func=mybir.ActivationFunctionType.Sqrt)

            # ---- Newton-Schulz (everything symmetric, so lhsT == operand) ---
            Y = smalls.tile([B, B], f32)
            nc.vector.tensor_scalar_mul(out=Y[:, :], in0=cov[:, :], scalar1=rnorm[:, :])
            Z = None
            for it in range(N_ITER):
                T = nsp.tile([B, B], f32, tag="T", name=f"T{it}")
                if it == 0:
                    # Z_0 = I  ->  T = 3I - Y
                    nc.vector.tensor_sub(out=T[:, :], in0=eye3[:, :], in1=Y[:, :])
                else:
                    zy_ps = pss.tile([B, B], f32, tag="pss", name=f"zy{it}")
                    nc.tensor.matmul(zy_ps[:, :], lhsT=Z[:, :], rhs=Y[:, :],
                                     start=True, stop=True)
                    nc.vector.tensor_sub(out=T[:, :], in0=eye3[:, :], in1=zy_ps[:, :])
                yn_ps = pss.tile([B, B], f32, tag="pss", name=f"yn{it}")
                nc.tensor.matmul(yn_ps[:, :], lhsT=Y[:, :], rhs=T[:, :],
                                 start=True, stop=True)
                Yn = nsp.tile([B, B], f32, tag="Y", name=f"Y{it}")
                nc.vector.tensor_scalar_mul(out=Yn[:, :], in0=yn_ps[:, :], scalar1=0.5)
                Zn = nsp.tile([B, B], f32, tag="Z", name=f"Z{it}")
                if it == 0:
                    nc.vector.tensor_scalar_mul(out=Zn[:, :], in0=T[:, :], scalar1=0.5)
                else:
                    zn_ps = pss.tile([B, B], f32, tag="pss", name=f"zn{it}")
                    nc.tensor.matmul(zn_ps[:, :], lhsT=T[:, :], rhs=Z[:, :],
                                     start=True, stop=True)
                    nc.vector.tensor_scalar_mul(out=Zn[:, :], in0=zn_ps[:, :],
                                                scalar1=0.5)
                Y, Z = Yn, Zn

            deconv = smalls.tile([B, B], f32)
            nc.vector.tensor_scalar_mul(out=deconv[:, :], in0=Z[:, :],
                                        scalar1=rqnorm[:, :])

            # ---- stacked bias dm2 = [D@mean; D@mean] via partition-offset
            # matmuls (no cross-partition SBUF DMAs needed)
            mean_sb = smalls.tile([B, 1], f32)
            nc.vector.tensor_scalar_mul(out=mean_sb[:, :], in0=ar_sb[0:B, B:B + 1],
                                        scalar1=inv_cnt)
            dm2_ps = pss.tile([128, 1], f32, tag="pss")
            nc.tensor.matmul(dm2_ps[0:B, :], lhsT=deconv[:, :], rhs=mean_sb[:, :],
                             start=True, stop=True)
            nc.tensor.matmul(dm2_ps[B:128, :], lhsT=deconv[:, :], rhs=mean_sb[:, :],
                             start=True, stop=True)
            dm2 = smalls.tile([128, 1], f32)
            nc.vector.tensor_copy(out=dm2[:, :], in_=dm2_ps[:, :])

            # ---- block-diagonal [[D,0],[0,D]]: D @ I into both diagonal
            # PSUM quadrants (out base-partition 0 and 64), off-diagonal
            # quadrants stay zero in a pre-zeroed SBUF tile
            dblk_ps = pss.tile([128, 128], f32, tag="pss")
            nc.tensor.matmul(dblk_ps[0:B, 0:B], lhsT=deconv[:, :], rhs=eye_sb[:, :],
                             start=True, stop=True)
            nc.tensor.matmul(dblk_ps[B:128, B:128], lhsT=deconv[:, :],
                             rhs=eye_sb[:, :], start=True, stop=True)
            dblk = smalls.tile([128, 128], f32r)
            zeros128 = smalls.tile([128, B], f32)
            nc.vector.memset(zeros128[:, :], 0.0)
            nc.vector.tensor_copy(out=dblk[0:B, B:128], in_=zeros128[0:B, :])
            nc.vector.tensor_copy(out=dblk[B:128, 0:B], in_=zeros128[B:128, :])
            nc.vector.tensor_copy(out=dblk[0:B, 0:B], in_=dblk_ps[0:B, 0:B])
            nc.vector.tensor_copy(out=dblk[B:128, B:128], in_=dblk_ps[B:128, B:128])

            # ---- whitening: y = Dblk @ S - dm2 ------------------------------
            # f32r matmuls: full PE streaming rate (f32 is 4 cycles/row and
            # paces the whole output phase). Output DMAs ride the ACT HWDGE
            # ring so they interleave with the input stream on the SP ring.
            NBLK = 512                      # one PSUM bank
            GRP = 4                         # blocks per output DMA (1 MiB)
            for jo in range(WIDE // (NBLK * GRP)):
                y_sb = outs.tile([128, NBLK * GRP], f32, tag="y", name=f"y{jo}")
                for ji in range(GRP):
                    j = jo * GRP + ji
                    w_ps = psw.tile([128, NBLK], f32, tag="w", name=f"w{j}")
                    nc.tensor.matmul(
                        w_ps[:, :], lhsT=dblk[:, :],
                        rhs=S[:, j * NBLK:(j + 1) * NBLK],
                        start=True, stop=True,
                    )
                    nc.vector.tensor_scalar(
                        out=y_sb[:, ji * NBLK:(ji + 1) * NBLK],
                        in0=w_ps[:, :], scalar1=dm2[:, :], scalar2=None,
                        op0=mybir.AluOpType.subtract,
                    )
                nc.scalar.dma_start(
                    out=out_ext[:, jo * NBLK * GRP:(jo + 1) * NBLK * GRP],
                    in_=y_sb[:, :],
                )

    nc.finalize()
    return nc


def _shard_inputs(x):
    """Build per-core input maps from the full (32,64,128,128) f32 tensor."""
    x = np.ascontiguousarray(x, dtype=np.float32)
    xr = x.reshape(N, C, HW)

    # global stride-9 sample gather, already transposed to [n_samples, 64]
    ls = np.arange(0, N * HW, STRIDE2, dtype=np.int64)
    ns_idx = ls // HW
    hw_idx = ls % HW
    xs_all = xr[ns_idx, :, hw_idx]          # [58255, 64]

    eye = np.eye(B, dtype=np.float32)
    in_maps = []
    for k in range(CORES):
        # stacked main input [128, 32768]
        x4 = x[NL * k:NL * (k + 1)].reshape(2, 2, C, HW)
        xp = np.ascontiguousarray(
            x4.transpose(0, 2, 1, 3).reshape(128, WIDE))

        # this core's slice of the global sample set
        s0 = -(-(LPC * k) // STRIDE2)        # ceil
        s1 = -(-(LPC * (k + 1)) // STRIDE2)
        m = s1 - s0
        xst = np.zeros((NCHUNK * 128, MCOL), dtype=np.float32)
        xst[:m, :B] = xs_all[s0:s1]
        xst[:m, B] = 1.0
        xst = np.ascontiguousarray(
            xst.reshape(NCHUNK, 128, MCOL).transpose(1, 0, 2)
            .reshape(128, NCHUNK * MCOL))

        in_maps.append({"xp": xp, "xst": xst, "eye": eye})
    return in_maps


def _unshard_output(results):
    y = np.empty((N, C, H, W), dtype=np.float32)
    for k in range(CORES):
        o = results[k]["out"].reshape(2, C, 2, HW)
        y[NL * k:NL * (k + 1)] = (
            o.transpose(0, 2, 1, 3).reshape(NL, C, H, W))
    return y


def kernel(x):
    from concourse.bass_utils import run_bass_kernel_spmd

    if "nc" not in _cached:
        _cached["nc"] = _build_nc()
    nc = _cached["nc"]

    in_maps = _shard_inputs(np.asarray(x))
    res = run_bass_kernel_spmd(nc, in_maps, core_ids=list(range(CORES)))
    _cached["last_results"] = res
    return _unshard_output(res.results)
